# revision 26
# baseline (speedup 1.0000x reference)
"""Trainium2 Bass kernel for nn_GCN_23029614641773.

The reference GCN operates on B independent 27-node graphs where every node of
graph i starts with the same feature vector x[i], and only node 0 of each graph
feeds the classifier head. Exploiting linearity of the edge aggregation, the
whole network collapses exactly (up to fp rounding order) to a per-sample MLP:

    y = x @ W0                                  # [B, 1024]
    s = lrelu(y + b0) + 2*lrelu(3y + b0) + lrelu(5y + b0)
      # node 1's in-neighbours {0,2,4,6} have in-degrees {1,3,3,5};
      # 2*lrelu(3y+b0) == lrelu(6y+2*b0) exactly (scaling by 2 is exact).
      # With b0 == 0 (spec fill): s == max(12y, 2.4y) exactly.
    t = s @ W1;  h = lrelu(t + b1)              # [B, 512]
    v = h @ W2;  g = lrelu(v + b2)              # [B, 256]
    out = g @ Wc + bc                           # [B, 1]

Sharding: pure data parallelism, batch split across 8 NeuronCores; each core
holds the full weight set.

Layout on device: activations kept transposed (features on partitions, batch
on the free dim) so every layer is matmul(out_T, lhsT=W_chunk, rhs=act_T) with
K accumulated in PSUM. x is transposed once on-chip via PE transposes.
"""

import os
from contextlib import ExitStack

import numpy as np

import concourse.bacc as bacc
import concourse.bass as bass
import concourse.mybir as mybir
import concourse.tile as tile
from concourse.bass_utils import run_bass_kernel_spmd

F32 = mybir.dt.float32
P = 128
N_CORES = 8
B_FULL = 2048
B = B_FULL // N_CORES  # 256 rows per core
D0, D1, D2, D3 = 1024, 1024, 512, 256
K0, M0 = D0 // P, D1 // P  # 8, 8
K1, M1 = D1 // P, D2 // P  # 8, 4
K2, M2 = D2 // P, D3 // P  # 4, 2
KC = D3 // P  # 2

NEG_SLOPE = 0.2
MM_DT = F32  # SBUF tile dtype for matmul operands
USE_F32R = True  # stream matmuls as float32r (4x faster on TRN2 PE)
F32R = mybir.dt.float32r


def _mm(ap):
    return ap.bitcast(F32R) if USE_F32R else ap


def _leaky(nc, out_ap, in_ap):
    # out = max(in, 0.2*in) == leaky_relu(in, 0.2), exact in fp32.
    # (Requires in_ap in SBUF: both operands feed one instruction.)
    nc.vector.scalar_tensor_tensor(
        out_ap, in_ap, 0.2, in_ap,
        mybir.AluOpType.mult, mybir.AluOpType.max,
    )


def _leaky_psum(nc, tmp_pool, out_ap, ps_ap):
    # leaky_relu straight out of PSUM: only one non-scalar input may read
    # PSUM, so stage 0.2*ps in SBUF then max against PSUM.
    t = tmp_pool.tile([ps_ap.partition_size(), ps_ap.free_size()], F32,
                      tag="lk")
    nc.vector.tensor_scalar_mul(t[:], ps_ap, 0.2)
    nc.vector.tensor_max(out_ap, ps_ap, t[:])


def _build(zero_bias: bool):
    nc = bacc.Bacc(
        "TRN2", target_bir_lowering=False, debug=False,
        enable_asserts=False, num_devices=N_CORES,
    )

    x_d = nc.dram_tensor("x", [B, D0], F32, kind="ExternalInput").ap()
    w0_d = nc.dram_tensor("W0", [D0, D1], F32, kind="ExternalInput").ap()
    b0_d = nc.dram_tensor("b0", [D1], F32, kind="ExternalInput").ap()
    w1_d = nc.dram_tensor("W1", [D1, D2], F32, kind="ExternalInput").ap()
    b1_d = nc.dram_tensor("b1", [D2], F32, kind="ExternalInput").ap()
    w2_d = nc.dram_tensor("W2", [D2, D3], F32, kind="ExternalInput").ap()
    b2_d = nc.dram_tensor("b2", [D3], F32, kind="ExternalInput").ap()
    wc_d = nc.dram_tensor("Wc", [D3, 1], F32, kind="ExternalInput").ap()
    bc_d = nc.dram_tensor("bc", [1], F32, kind="ExternalInput").ap()
    eye_d = nc.dram_tensor("eye", [P, P], F32, kind="ExternalInput").ap()
    out_d = nc.dram_tensor("out", [1, B], F32, kind="ExternalOutput").ap()

    with ExitStack() as ctx:
        tc = ctx.enter_context(tile.TileContext(nc))
        const = ctx.enter_context(tc.tile_pool(name="const", bufs=1))
        xrow_p = ctx.enter_context(tc.tile_pool(name="xrow", bufs=2))
        xt_p = ctx.enter_context(tc.tile_pool(name="xt", bufs=K0))
        w0_p = ctx.enter_context(tc.tile_pool(name="w0", bufs=K0))
        w1_p = ctx.enter_context(tc.tile_pool(name="w1", bufs=K1))
        w2_p = ctx.enter_context(tc.tile_pool(name="w2", bufs=K2))
        wc_p = ctx.enter_context(tc.tile_pool(name="wc", bufs=1))
        s_p = ctx.enter_context(tc.tile_pool(name="s", bufs=K1))
        h_p = ctx.enter_context(tc.tile_pool(name="h", bufs=K2))
        g_p = ctx.enter_context(tc.tile_pool(name="g", bufs=KC))
        tmp_p = ctx.enter_context(tc.tile_pool(name="tmp", bufs=4))
        out_p = ctx.enter_context(tc.tile_pool(name="outp", bufs=1))
        ps_p = ctx.enter_context(tc.tile_pool(name="ps", bufs=6, space="PSUM"))
        cls_ps = ctx.enter_context(tc.tile_pool(name="cls", bufs=1, space="PSUM"))

        rings = [nc.sync, nc.scalar]

        # leaky-relu slope as a per-partition alpha vector for ACT Prelu
        alt = const.tile([P, 1], F32, tag="alt")
        nc.vector.memset(alt[:], NEG_SLOPE)

        # ---- DMA order = HBM arrival order, alternating the two HWDGE
        # rings: eye + x first (transposes), then W0 as column blocks
        # (each m-group of layer 1 streams in behind one block), W1, W2, Wc.
        eye = const.tile([P, P], F32, tag="eye")
        nc.scalar.dma_start(eye[:], eye_d)
        xr = []
        for r in range(B // P):
            t = xrow_p.tile([P, D0], F32)
            rings[r % 2].dma_start(t[:], x_d[r * P:(r + 1) * P, :])
            xr.append(t)

        # W as contraction-chunk row tiles: chunk c = W[c*128:(c+1)*128, :]
        # (contiguous rows -> cheap DMA descriptors); lhsT for (c, m) is
        # chunk_c[:, m*128:(m+1)*128]
        def row_chunks(pool, w_dram, K, N):
            chunks = []
            for c in range(K):
                t = pool.tile([P, N], F32, tag="w",
                              name=f"wchk_{w_dram.tensor.name}_{c}")
                nc.sync.dma_start(_mm(t[:]), _mm(w_dram[c * P:(c + 1) * P, :]))
                chunks.append(t)
            return chunks

        w0 = row_chunks(w0_p, w0_d, K0, D1)
        w1 = row_chunks(w1_p, w1_d, K1, D2)
        w2 = row_chunks(w2_p, w2_d, K2, D3)
        wc = wc_p.tile([P, KC], F32)
        nc.sync.dma_start(_mm(wc[:]), _mm(wc_d.rearrange("(c p) j -> p c j", p=P)))

        if not zero_bias:
            b0t = const.tile([P, M0], F32, tag="b0t")
            nc.scalar.dma_start(b0t[:], b0_d.rearrange("(c p) -> p c", p=P))
            b1t = const.tile([P, M1], F32, tag="b1t")
            nc.scalar.dma_start(b1t[:], b1_d.rearrange("(c p) -> p c", p=P))
            b2t = const.tile([P, M2], F32, tag="b2t")
            nc.scalar.dma_start(b2t[:], b2_d.rearrange("(c p) -> p c", p=P))
            bct = const.tile([1, 1], F32, tag="bct")
            nc.scalar.dma_start(bct[:], bc_d.rearrange("(a b) -> a b", a=1))
            b0t2 = const.tile([P, M0], F32, tag="b0t2")
            nc.vector.tensor_scalar_mul(b0t2[:], b0t[:], 2.0)

        # ---- transpose x: [256, 1024] -> 8 tiles [128, 256] ----
        xt = []
        for k in range(K0):
            xtk = xt_p.tile([P, B], F32, tag="xt", name=f"xt_{k}")
            for r in range(B // P):
                pt = ps_p.tile([P, P], F32, tag="ps", name=f"pt_{k}_{r}")
                nc.tensor.transpose(pt[:], xr[r][:, k * P:(k + 1) * P], eye[:])
                nc.vector.tensor_copy(_mm(xtk[:, r * P:(r + 1) * P]), pt[:])
            xt.append(xtk)

        PRELU = mybir.ActivationFunctionType.Prelu

        def matmul_group(ps, chunks, m, rhs_tiles, K):
            for c in range(K):
                nc.tensor.matmul(
                    ps[:], lhsT=_mm(chunks[c][:, m * P:(m + 1) * P]),
                    rhs=_mm(rhs_tiles[c][:]),
                    start=(c == 0), stop=(c == K - 1),
                )

        # ---- layer 1: y[m] = sum_c W0[c,m].T @ xT[c];
        #      s = 12*lrelu(y) = Prelu(12*y) exactly (zero bias) ----
        s_tiles = []
        for m in range(M0):
            ps = ps_p.tile([P, B], F32, tag="ps", name=f"ps1_{m}")
            matmul_group(ps, w0, m, xt, K0)
            s = s_p.tile([P, B], F32, tag="s", name=f"s_{m}")
            if zero_bias:
                nc.scalar.activation(_mm(s[:]), ps[:], PRELU,
                                     scale=12.0, alpha=alt[:])
            else:
                first = True
                for scale, bias in ((1.0, b0t[:, m:m + 1]), (6.0, b0t2[:, m:m + 1]),
                                    (5.0, b0t[:, m:m + 1])):
                    l = tmp_p.tile([P, B], F32, tag="l", name=f"l_{m}")
                    nc.scalar.activation(l[:], ps[:], PRELU,
                                         scale=scale, bias=bias, alpha=alt[:])
                    if first:
                        nc.vector.tensor_copy(_mm(s[:]), l[:])
                        first = False
                    else:
                        nc.vector.tensor_add(_mm(s[:]), _mm(s[:]), l[:])
            s_tiles.append(s)

        # ---- layer 2: t[m] = sum_c W1[c,m].T @ s[c]; h = lrelu(t + b1) ----
        h_tiles = []
        for m in range(M1):
            ps = ps_p.tile([P, B], F32, tag="ps", name=f"ps2_{m}")
            matmul_group(ps, w1, m, s_tiles, K1)
            h = h_p.tile([P, B], F32, tag="h", name=f"h_{m}")
            if zero_bias:
                nc.scalar.activation(_mm(h[:]), ps[:], PRELU, alpha=alt[:])
            else:
                nc.scalar.activation(_mm(h[:]), ps[:], PRELU,
                                     bias=b1t[:, m:m + 1], alpha=alt[:])
            h_tiles.append(h)

        # ---- layer 3: v[m] = sum_c W2[c,m].T @ h[c]; g = lrelu(v + b2) ----
        g_tiles = []
        for m in range(M2):
            ps = ps_p.tile([P, B], F32, tag="ps", name=f"ps3_{m}")
            matmul_group(ps, w2, m, h_tiles, K2)
            g = g_p.tile([P, B], F32, tag="g", name=f"g_{m}")
            if zero_bias:
                nc.scalar.activation(_mm(g[:]), ps[:], PRELU, alpha=alt[:])
            else:
                nc.scalar.activation(_mm(g[:]), ps[:], PRELU,
                                     bias=b2t[:, m:m + 1], alpha=alt[:])
            g_tiles.append(g)

        # ---- classifier: out[1, B] = sum_c Wc[c].T @ g[c] (+ bc) ----
        po = cls_ps.tile([1, B], F32)
        for c in range(KC):
            nc.tensor.matmul(
                po[:], lhsT=_mm(wc[:, c:c + 1]), rhs=_mm(g_tiles[c][:]),
                start=(c == 0), stop=(c == KC - 1),
            )
        ob = out_p.tile([1, B], F32)
        if zero_bias:
            nc.vector.tensor_copy(ob[:], po[:])
        else:
            nc.vector.tensor_scalar_add(ob[:], po[:], bct[:, 0:1])
        nc.sync.dma_start(out_d, ob[:])

    nc.compile()
    return nc


_CACHE = {}


def _get_nc(zero_bias: bool):
    if zero_bias not in _CACHE:
        _CACHE[zero_bias] = _build(zero_bias)
    return _CACHE[zero_bias]


def _run(inputs, trace=False, **kw):
    def f32(a):
        return np.ascontiguousarray(np.asarray(a), dtype=np.float32)

    x = f32(inputs["x"])
    W0, b0 = f32(inputs["W0"]), f32(inputs["b0"])
    W1, b1 = f32(inputs["W1"]), f32(inputs["b1"])
    W2, b2 = f32(inputs["W2"]), f32(inputs["b2"])
    Wc, bc = f32(inputs["Wc"]), f32(inputs["bc"])
    zero_bias = not (b0.any() or b1.any() or b2.any() or bc.any())
    nc = _get_nc(zero_bias)

    eye = np.eye(P, dtype=np.float32)
    in_maps = []
    for i in range(N_CORES):
        in_maps.append({
            "x": x[i * B:(i + 1) * B],
            "W0": W0, "b0": b0, "W1": W1, "b1": b1,
            "W2": W2, "b2": b2, "Wc": Wc, "bc": bc,
            "eye": eye,
        })
    res = run_bass_kernel_spmd(nc, in_maps, list(range(N_CORES)),
                               trace=trace, **kw)
    out = np.empty((B_FULL, 1), dtype=np.float32)
    for i in range(N_CORES):
        out[i * B:(i + 1) * B, 0] = res.results[i]["out"][0]
    return out, res


def kernel(**inputs) -> np.ndarray:
    out, _ = _run(inputs)
    return out


# revision 27
# speedup vs baseline: 1.0249x; 1.0249x over previous
"""Trainium2 Bass kernel for nn_GCN_23029614641773.

The reference GCN operates on B independent 27-node graphs where every node of
graph i starts with the same feature vector x[i], and only node 0 of each graph
feeds the classifier head. Exploiting linearity of the edge aggregation, the
whole network collapses exactly (up to fp rounding order) to a per-sample MLP:

    y = x @ W0                                  # [B, 1024]
    s = lrelu(y + b0) + 2*lrelu(3y + b0) + lrelu(5y + b0)
      # node 1's in-neighbours {0,2,4,6} have in-degrees {1,3,3,5};
      # 2*lrelu(3y+b0) == lrelu(6y+2*b0) exactly (scaling by 2 is exact).
      # With b0 == 0 (spec fill): s == max(12y, 2.4y) exactly.
    t = s @ W1;  h = lrelu(t + b1)              # [B, 512]
    v = h @ W2;  g = lrelu(v + b2)              # [B, 256]
    out = g @ Wc + bc                           # [B, 1]

Sharding: pure data parallelism, batch split across 8 NeuronCores; each core
holds the full weight set.

Layout on device: activations kept transposed (features on partitions, batch
on the free dim) so every layer is matmul(out_T, lhsT=W_chunk, rhs=act_T) with
K accumulated in PSUM. x is transposed once on-chip via PE transposes.
"""

import os
from contextlib import ExitStack

import numpy as np

import concourse.bacc as bacc
import concourse.bass as bass
import concourse.mybir as mybir
import concourse.tile as tile
from concourse.bass_utils import run_bass_kernel_spmd

F32 = mybir.dt.float32
P = 128
N_CORES = 8
B_FULL = 2048
B = B_FULL // N_CORES  # 256 rows per core
D0, D1, D2, D3 = 1024, 1024, 512, 256
K0, M0 = D0 // P, D1 // P  # 8, 8
K1, M1 = D1 // P, D2 // P  # 8, 4
K2, M2 = D2 // P, D3 // P  # 4, 2
KC = D3 // P  # 2

NEG_SLOPE = 0.2
MM_DT = F32  # SBUF tile dtype for matmul operands
USE_F32R = True  # stream matmuls as float32r (4x faster on TRN2 PE)
F32R = mybir.dt.float32r


def _mm(ap):
    return ap.bitcast(F32R) if USE_F32R else ap


def _leaky(nc, out_ap, in_ap):
    # out = max(in, 0.2*in) == leaky_relu(in, 0.2), exact in fp32.
    # (Requires in_ap in SBUF: both operands feed one instruction.)
    nc.vector.scalar_tensor_tensor(
        out_ap, in_ap, 0.2, in_ap,
        mybir.AluOpType.mult, mybir.AluOpType.max,
    )


def _leaky_psum(nc, tmp_pool, out_ap, ps_ap):
    # leaky_relu straight out of PSUM: only one non-scalar input may read
    # PSUM, so stage 0.2*ps in SBUF then max against PSUM.
    t = tmp_pool.tile([ps_ap.partition_size(), ps_ap.free_size()], F32,
                      tag="lk")
    nc.vector.tensor_scalar_mul(t[:], ps_ap, 0.2)
    nc.vector.tensor_max(out_ap, ps_ap, t[:])


def _build(zero_bias: bool):
    nc = bacc.Bacc(
        "TRN2", target_bir_lowering=False, debug=False,
        enable_asserts=False, num_devices=N_CORES,
    )

    x_d = nc.dram_tensor("x", [B, D0], F32, kind="ExternalInput").ap()
    w0_d = nc.dram_tensor("W0", [D0, D1], F32, kind="ExternalInput").ap()
    b0_d = nc.dram_tensor("b0", [D1], F32, kind="ExternalInput").ap()
    w1_d = nc.dram_tensor("W1", [D1, D2], F32, kind="ExternalInput").ap()
    b1_d = nc.dram_tensor("b1", [D2], F32, kind="ExternalInput").ap()
    w2_d = nc.dram_tensor("W2", [D2, D3], F32, kind="ExternalInput").ap()
    b2_d = nc.dram_tensor("b2", [D3], F32, kind="ExternalInput").ap()
    wc_d = nc.dram_tensor("Wc", [D3, 1], F32, kind="ExternalInput").ap()
    bc_d = nc.dram_tensor("bc", [1], F32, kind="ExternalInput").ap()
    eye_d = nc.dram_tensor("eye", [P, P], F32, kind="ExternalInput").ap()
    out_d = nc.dram_tensor("out", [1, B], F32, kind="ExternalOutput").ap()

    with ExitStack() as ctx:
        tc = ctx.enter_context(tile.TileContext(nc))
        const = ctx.enter_context(tc.tile_pool(name="const", bufs=1))
        xrow_p = ctx.enter_context(tc.tile_pool(name="xrow", bufs=2))
        xt_p = ctx.enter_context(tc.tile_pool(name="xt", bufs=K0))
        w0_p = ctx.enter_context(tc.tile_pool(name="w0", bufs=K0))
        w1_p = ctx.enter_context(tc.tile_pool(name="w1", bufs=K1))
        w2_p = ctx.enter_context(tc.tile_pool(name="w2", bufs=K2))
        wc_p = ctx.enter_context(tc.tile_pool(name="wc", bufs=1))
        s_p = ctx.enter_context(tc.tile_pool(name="s", bufs=K1))
        h_p = ctx.enter_context(tc.tile_pool(name="h", bufs=K2))
        g_p = ctx.enter_context(tc.tile_pool(name="g", bufs=KC))
        tmp_p = ctx.enter_context(tc.tile_pool(name="tmp", bufs=4))
        out_p = ctx.enter_context(tc.tile_pool(name="outp", bufs=1))
        ps_p = ctx.enter_context(tc.tile_pool(name="ps", bufs=7, space="PSUM"))
        cls_ps = ctx.enter_context(tc.tile_pool(name="cls", bufs=1, space="PSUM"))

        rings = [nc.sync, nc.scalar]

        # leaky-relu slope as a per-partition alpha vector for ACT Prelu
        alt = const.tile([P, 1], F32, tag="alt")
        nc.vector.memset(alt[:], NEG_SLOPE)

        # ---- DMA order = HBM arrival order, alternating the two HWDGE
        # rings: eye + x first (transposes), then W0 as column blocks
        # (each m-group of layer 1 streams in behind one block), W1, W2, Wc.
        eye = const.tile([P, P], F32, tag="eye")
        nc.scalar.dma_start(eye[:], eye_d)
        xr = []
        for r in range(B // P):
            t = xrow_p.tile([P, D0], F32)
            rings[r % 2].dma_start(t[:], x_d[r * P:(r + 1) * P, :])
            xr.append(t)

        # W as contraction-chunk row tiles: chunk c = W[c*128:(c+1)*128, :]
        # (contiguous rows -> cheap DMA descriptors); lhsT for (c, m) is
        # chunk_c[:, m*128:(m+1)*128]
        def row_chunks(pool, w_dram, K, N):
            chunks = []
            for c in range(K):
                t = pool.tile([P, N], F32, tag="w",
                              name=f"wchk_{w_dram.tensor.name}_{c}")
                nc.sync.dma_start(_mm(t[:]), _mm(w_dram[c * P:(c + 1) * P, :]))
                chunks.append(t)
            return chunks

        w0 = row_chunks(w0_p, w0_d, K0, D1)
        w1 = row_chunks(w1_p, w1_d, K1, D2)
        w2 = row_chunks(w2_p, w2_d, K2, D3)
        wc = wc_p.tile([P, KC], F32)
        nc.sync.dma_start(_mm(wc[:]), _mm(wc_d.rearrange("(c p) j -> p c j", p=P)))

        if not zero_bias:
            b0t = const.tile([P, M0], F32, tag="b0t")
            nc.scalar.dma_start(b0t[:], b0_d.rearrange("(c p) -> p c", p=P))
            b1t = const.tile([P, M1], F32, tag="b1t")
            nc.scalar.dma_start(b1t[:], b1_d.rearrange("(c p) -> p c", p=P))
            b2t = const.tile([P, M2], F32, tag="b2t")
            nc.scalar.dma_start(b2t[:], b2_d.rearrange("(c p) -> p c", p=P))
            bct = const.tile([1, 1], F32, tag="bct")
            nc.scalar.dma_start(bct[:], bc_d.rearrange("(a b) -> a b", a=1))
            b0t2 = const.tile([P, M0], F32, tag="b0t2")
            nc.vector.tensor_scalar_mul(b0t2[:], b0t[:], 2.0)

        # ---- transpose x: [256, 1024] -> 8 tiles [128, 256] ----
        xt = []
        for k in range(K0):
            xtk = xt_p.tile([P, B], F32, tag="xt", name=f"xt_{k}")
            for r in range(B // P):
                pt = ps_p.tile([P, P], F32, tag="ps", name=f"pt_{k}_{r}")
                nc.tensor.transpose(pt[:], xr[r][:, k * P:(k + 1) * P], eye[:])
                nc.vector.tensor_copy(_mm(xtk[:, r * P:(r + 1) * P]), pt[:])
            xt.append(xtk)

        PRELU = mybir.ActivationFunctionType.Prelu

        def matmul_group(ps, chunks, m, rhs_tiles, K):
            for c in range(K):
                nc.tensor.matmul(
                    ps[:], lhsT=_mm(chunks[c][:, m * P:(m + 1) * P]),
                    rhs=_mm(rhs_tiles[c][:]),
                    start=(c == 0), stop=(c == K - 1),
                )

        # ---- layer 1: y[m] = sum_c W0[c,m].T @ xT[c];
        #      s = 12*lrelu(y) = Prelu(12*y) exactly (zero bias) ----
        s_tiles = []
        for m in range(M0):
            ps = ps_p.tile([P, B], F32, tag="ps", name=f"ps1_{m}")
            matmul_group(ps, w0, m, xt, K0)
            s = s_p.tile([P, B], F32, tag="s", name=f"s_{m}")
            if zero_bias:
                nc.scalar.activation(_mm(s[:]), ps[:], PRELU,
                                     scale=12.0, alpha=alt[:])
            else:
                first = True
                for scale, bias in ((1.0, b0t[:, m:m + 1]), (6.0, b0t2[:, m:m + 1]),
                                    (5.0, b0t[:, m:m + 1])):
                    l = tmp_p.tile([P, B], F32, tag="l", name=f"l_{m}")
                    nc.scalar.activation(l[:], ps[:], PRELU,
                                         scale=scale, bias=bias, alpha=alt[:])
                    if first:
                        nc.vector.tensor_copy(_mm(s[:]), l[:])
                        first = False
                    else:
                        nc.vector.tensor_add(_mm(s[:]), _mm(s[:]), l[:])
            s_tiles.append(s)

        # ---- layer 2: t[m] = sum_c W1[c,m].T @ s[c]; h = lrelu(t + b1) ----
        h_tiles = []
        for m in range(M1):
            ps = ps_p.tile([P, B], F32, tag="ps", name=f"ps2_{m}")
            matmul_group(ps, w1, m, s_tiles, K1)
            h = h_p.tile([P, B], F32, tag="h", name=f"h_{m}")
            if zero_bias:
                nc.scalar.activation(_mm(h[:]), ps[:], PRELU, alpha=alt[:])
            else:
                nc.scalar.activation(_mm(h[:]), ps[:], PRELU,
                                     bias=b1t[:, m:m + 1], alpha=alt[:])
            h_tiles.append(h)

        # ---- layer 3: v[m] = sum_c W2[c,m].T @ h[c]; g = lrelu(v + b2) ----
        g_tiles = []
        for m in range(M2):
            ps = ps_p.tile([P, B], F32, tag="ps", name=f"ps3_{m}")
            matmul_group(ps, w2, m, h_tiles, K2)
            g = g_p.tile([P, B], F32, tag="g", name=f"g_{m}")
            if zero_bias:
                nc.scalar.activation(_mm(g[:]), ps[:], PRELU, alpha=alt[:])
            else:
                nc.scalar.activation(_mm(g[:]), ps[:], PRELU,
                                     bias=b2t[:, m:m + 1], alpha=alt[:])
            g_tiles.append(g)

        # ---- classifier: out[1, B] = sum_c Wc[c].T @ g[c] (+ bc) ----
        po = cls_ps.tile([1, B], F32)
        for c in range(KC):
            nc.tensor.matmul(
                po[:], lhsT=_mm(wc[:, c:c + 1]), rhs=_mm(g_tiles[c][:]),
                start=(c == 0), stop=(c == KC - 1),
            )
        ob = out_p.tile([1, B], F32)
        if zero_bias:
            nc.vector.tensor_copy(ob[:], po[:])
        else:
            nc.vector.tensor_scalar_add(ob[:], po[:], bct[:, 0:1])
        nc.sync.dma_start(out_d, ob[:])

    nc.compile()
    return nc


_CACHE = {}


def _get_nc(zero_bias: bool):
    if zero_bias not in _CACHE:
        _CACHE[zero_bias] = _build(zero_bias)
    return _CACHE[zero_bias]


def _run(inputs, trace=False, **kw):
    def f32(a):
        return np.ascontiguousarray(np.asarray(a), dtype=np.float32)

    x = f32(inputs["x"])
    W0, b0 = f32(inputs["W0"]), f32(inputs["b0"])
    W1, b1 = f32(inputs["W1"]), f32(inputs["b1"])
    W2, b2 = f32(inputs["W2"]), f32(inputs["b2"])
    Wc, bc = f32(inputs["Wc"]), f32(inputs["bc"])
    zero_bias = not (b0.any() or b1.any() or b2.any() or bc.any())
    nc = _get_nc(zero_bias)

    eye = np.eye(P, dtype=np.float32)
    in_maps = []
    for i in range(N_CORES):
        in_maps.append({
            "x": x[i * B:(i + 1) * B],
            "W0": W0, "b0": b0, "W1": W1, "b1": b1,
            "W2": W2, "b2": b2, "Wc": Wc, "bc": bc,
            "eye": eye,
        })
    res = run_bass_kernel_spmd(nc, in_maps, list(range(N_CORES)),
                               trace=trace, **kw)
    out = np.empty((B_FULL, 1), dtype=np.float32)
    for i in range(N_CORES):
        out[i * B:(i + 1) * B, 0] = res.results[i]["out"][0]
    return out, res


def kernel(**inputs) -> np.ndarray:
    out, _ = _run(inputs)
    return out


# revision 41
# speedup vs baseline: 1.0864x; 1.0600x over previous
"""Trainium2 Bass kernel for nn_GCN_23029614641773.

The reference GCN operates on B independent 27-node graphs where every node of
graph i starts with the same feature vector x[i], and only node 0 of each graph
feeds the classifier head. Exploiting linearity of the edge aggregation, the
whole network collapses exactly (up to fp rounding order) to a per-sample MLP:

    y = x @ W0                                  # [B, 1024]
    s = lrelu(y + b0) + 2*lrelu(3y + b0) + lrelu(5y + b0)
      # node 1's in-neighbours {0,2,4,6} have in-degrees {1,3,3,5};
      # 2*lrelu(3y+b0) == lrelu(6y+2*b0) exactly (scaling by 2 is exact).
      # With b0 == 0 (spec fill): s == max(12y, 2.4y) exactly.
    t = s @ W1;  h = lrelu(t + b1)              # [B, 512]
    v = h @ W2;  g = lrelu(v + b2)              # [B, 256]
    out = g @ Wc + bc                           # [B, 1]

Sharding: pure data parallelism, batch split across 8 NeuronCores; each core
holds the full weight set.

Layout on device: activations kept transposed (features on partitions, batch
on the free dim) so every layer is matmul(out_T, lhsT=W_chunk, rhs=act_T) with
K accumulated in PSUM. x is transposed once on-chip via PE transposes.
"""

from contextlib import ExitStack

import numpy as np

import concourse.bacc as bacc
import concourse.mybir as mybir
import concourse.tile as tile
from concourse.bass_utils import run_bass_kernel_spmd

F32 = mybir.dt.float32
P = 128
N_CORES = 8
B_FULL = 2048
B = B_FULL // N_CORES  # 256 rows per core
D0, D1, D2, D3 = 1024, 1024, 512, 256
K0, M0 = D0 // P, D1 // P  # 8, 8
K1, M1 = D1 // P, D2 // P  # 8, 4
K2, M2 = D2 // P, D3 // P  # 4, 2
KC = D3 // P  # 2

NEG_SLOPE = 0.2
USE_F32R = True  # stream matmuls as float32r (4x faster on TRN2 PE)
F32R = mybir.dt.float32r


def _mm(ap):
    return ap.bitcast(F32R) if USE_F32R else ap


def _build(zero_bias: bool):
    nc = bacc.Bacc(
        "TRN2", target_bir_lowering=False, debug=False,
        enable_asserts=False, num_devices=1,
    )

    x_d = nc.dram_tensor("x", [B, D0], F32, kind="ExternalInput").ap()
    w0_d = nc.dram_tensor("W0", [D0, D1], F32, kind="ExternalInput").ap()
    b0_d = nc.dram_tensor("b0", [D1], F32, kind="ExternalInput").ap()
    w1_d = nc.dram_tensor("W1", [D1, D2], F32, kind="ExternalInput").ap()
    b1_d = nc.dram_tensor("b1", [D2], F32, kind="ExternalInput").ap()
    w2_d = nc.dram_tensor("W2", [D2, D3], F32, kind="ExternalInput").ap()
    b2_d = nc.dram_tensor("b2", [D3], F32, kind="ExternalInput").ap()
    wc_d = nc.dram_tensor("Wc", [D3, 1], F32, kind="ExternalInput").ap()
    bc_d = nc.dram_tensor("bc", [1], F32, kind="ExternalInput").ap()
    eye_d = nc.dram_tensor("eye", [P, P], F32, kind="ExternalInput").ap()
    out_d = nc.dram_tensor("out", [1, B], F32, kind="ExternalOutput").ap()

    with ExitStack() as ctx:
        tc = ctx.enter_context(tile.TileContext(nc))
        const = ctx.enter_context(tc.tile_pool(name="const", bufs=1))
        xrow_p = ctx.enter_context(tc.tile_pool(name="xrow", bufs=2))
        xt_p = ctx.enter_context(tc.tile_pool(name="xt", bufs=K0))
        w0_p = ctx.enter_context(tc.tile_pool(name="w0", bufs=K0 // 2))
        w1_p = ctx.enter_context(tc.tile_pool(name="w1", bufs=K1 // 2))
        w2_p = ctx.enter_context(tc.tile_pool(name="w2", bufs=K2 // 2))
        wc_p = ctx.enter_context(tc.tile_pool(name="wc", bufs=1))
        s_p = ctx.enter_context(tc.tile_pool(name="s", bufs=K1))
        h_p = ctx.enter_context(tc.tile_pool(name="h", bufs=K2))
        g_p = ctx.enter_context(tc.tile_pool(name="g", bufs=KC))
        tmp_p = ctx.enter_context(tc.tile_pool(name="tmp", bufs=4))
        out_p = ctx.enter_context(tc.tile_pool(name="outp", bufs=1))
        ps_p = ctx.enter_context(tc.tile_pool(name="ps", bufs=7, space="PSUM"))
        cls_ps = ctx.enter_context(tc.tile_pool(name="cls", bufs=1, space="PSUM"))

        # leaky-relu slope as a per-partition alpha vector for ACT Prelu
        alt = const.tile([P, 1], F32, tag="alt")
        nc.vector.memset(alt[:], NEG_SLOPE)

        # ---- DMA order = HBM arrival order: eye + x first (feed the
        # transposes), then W0 (gates layer 1), W1, W2, Wc. All big loads on
        # the sync HWDGE ring; scalar ring stays free for activations. ----
        eye = const.tile([P, P], F32, tag="eye")
        nc.scalar.dma_start(eye[:], eye_d)
        xr = []
        xpair = xrow_p.tile([P, 2 * D0], F32, tag="xr", name="xpair")
        nc.sync.dma_start(xpair[:], x_d.rearrange("(c p) m -> p c m", p=P))
        for r in range(B // P):
            xr.append(xpair[:, r * D0:(r + 1) * D0])

        # W as contraction-chunk row tiles: chunk c = W[c*128:(c+1)*128, :]
        # (contiguous rows -> cheap DMA descriptors); lhsT for (c, m) is
        # chunk_c[:, m*128:(m+1)*128]
        def row_chunks(pool, w_dram, K, N):
            # pairs of contraction chunks per DMA (halves the issue count;
            # per-partition runs stay contiguous at N*4 bytes)
            chunks = []
            G = 2
            for i in range(K // G):
                t = pool.tile([P, G * N], F32, tag="w",
                              name=f"wgrp_{w_dram.tensor.name}_{i}")
                src_ap = w_dram[G * i * P:(G * i + G) * P, :].rearrange(
                    "(c p) m -> p c m", p=P)
                nc.sync.dma_start(_mm(t[:]), _mm(src_ap))
                for j in range(G):
                    chunks.append(t[:, j * N:(j + 1) * N])
            return chunks

        w0 = row_chunks(w0_p, w0_d, K0, D1)
        w1 = row_chunks(w1_p, w1_d, K1, D2)
        w2 = row_chunks(w2_p, w2_d, K2, D3)
        wc = wc_p.tile([P, KC], F32)
        nc.sync.dma_start(_mm(wc[:]), _mm(wc_d.rearrange("(c p) j -> p c j", p=P)))

        if not zero_bias:
            b0t = const.tile([P, M0], F32, tag="b0t")
            nc.scalar.dma_start(b0t[:], b0_d.rearrange("(c p) -> p c", p=P))
            b1t = const.tile([P, M1], F32, tag="b1t")
            nc.scalar.dma_start(b1t[:], b1_d.rearrange("(c p) -> p c", p=P))
            b2t = const.tile([P, M2], F32, tag="b2t")
            nc.scalar.dma_start(b2t[:], b2_d.rearrange("(c p) -> p c", p=P))
            bct = const.tile([1, 1], F32, tag="bct")
            nc.scalar.dma_start(bct[:], bc_d.rearrange("(a b) -> a b", a=1))
            b0t2 = const.tile([P, M0], F32, tag="b0t2")
            nc.vector.tensor_scalar_mul(b0t2[:], b0t[:], 2.0)

        # ---- transpose x: [256, 1024] -> 8 tiles [128, 256] ----
        xt = []
        for k in range(K0):
            xtk = xt_p.tile([P, B], F32, tag="xt", name=f"xt_{k}")
            for r in range(B // P):
                pt = ps_p.tile([P, P], F32, tag="ps", name=f"pt_{k}_{r}")
                nc.tensor.transpose(pt[:], xr[r][:, k * P:(k + 1) * P], eye[:])
                nc.vector.tensor_copy(_mm(xtk[:, r * P:(r + 1) * P]), pt[:])
            xt.append(xtk)

        PRELU = mybir.ActivationFunctionType.Prelu

        def matmul_group(ps, chunks, m, rhs_tiles, K):
            for c in range(K):
                nc.tensor.matmul(
                    ps[:], lhsT=_mm(chunks[c][:, m * P:(m + 1) * P]),
                    rhs=_mm(rhs_tiles[c][:]),
                    start=(c == 0), stop=(c == K - 1),
                )

        # ---- layer 1: y[m] = sum_c W0[c,m].T @ xT[c];
        #      s = 12*lrelu(y) = Prelu(12*y) exactly (zero bias) ----
        s_tiles = []
        for m in range(M0):
            ps = ps_p.tile([P, B], F32, tag="ps", name=f"ps1_{m}")
            matmul_group(ps, w0, m, xt, K0)
            s = s_p.tile([P, B], F32, tag="s", name=f"s_{m}")
            if zero_bias:
                nc.scalar.activation(_mm(s[:]), ps[:], PRELU,
                                     scale=12.0, alpha=alt[:])
            else:
                first = True
                for scale, bias in ((1.0, b0t[:, m:m + 1]), (6.0, b0t2[:, m:m + 1]),
                                    (5.0, b0t[:, m:m + 1])):
                    l = tmp_p.tile([P, B], F32, tag="l", name=f"l_{m}")
                    nc.scalar.activation(l[:], ps[:], PRELU,
                                         scale=scale, bias=bias, alpha=alt[:])
                    if first:
                        nc.vector.tensor_copy(_mm(s[:]), l[:])
                        first = False
                    else:
                        nc.vector.tensor_add(_mm(s[:]), _mm(s[:]), l[:])
            s_tiles.append(s)

        # ---- layer 2: t[m] = sum_c W1[c,m].T @ s[c]; h = lrelu(t + b1) ----
        h_tiles = []
        for m in range(M1):
            ps = ps_p.tile([P, B], F32, tag="ps", name=f"ps2_{m}")
            matmul_group(ps, w1, m, s_tiles, K1)
            h = h_p.tile([P, B], F32, tag="h", name=f"h_{m}")
            if zero_bias:
                nc.scalar.activation(_mm(h[:]), ps[:], PRELU, alpha=alt[:])
            else:
                nc.scalar.activation(_mm(h[:]), ps[:], PRELU,
                                     bias=b1t[:, m:m + 1], alpha=alt[:])
            h_tiles.append(h)

        # ---- layer 3: v[m] = sum_c W2[c,m].T @ h[c]; g = lrelu(v + b2) ----
        g_tiles = []
        for m in range(M2):
            ps = ps_p.tile([P, B], F32, tag="ps", name=f"ps3_{m}")
            matmul_group(ps, w2, m, h_tiles, K2)
            g = g_p.tile([P, B], F32, tag="g", name=f"g_{m}")
            if zero_bias:
                nc.scalar.activation(_mm(g[:]), ps[:], PRELU, alpha=alt[:])
            else:
                nc.scalar.activation(_mm(g[:]), ps[:], PRELU,
                                     bias=b2t[:, m:m + 1], alpha=alt[:])
            g_tiles.append(g)

        # ---- classifier: out[1, B] = sum_c Wc[c].T @ g[c] (+ bc) ----
        po = cls_ps.tile([1, B], F32)
        for c in range(KC):
            nc.tensor.matmul(
                po[:], lhsT=_mm(wc[:, c:c + 1]), rhs=_mm(g_tiles[c][:]),
                start=(c == 0), stop=(c == KC - 1),
            )
        ob = out_p.tile([1, B], F32)
        if zero_bias:
            nc.vector.tensor_copy(ob[:], po[:])
        else:
            nc.vector.tensor_scalar_add(ob[:], po[:], bct[:, 0:1])
        nc.sync.dma_start(out_d, ob[:])

    nc.compile()
    return nc


_CACHE = {}


def _get_nc(zero_bias: bool):
    if zero_bias not in _CACHE:
        _CACHE[zero_bias] = _build(zero_bias)
    return _CACHE[zero_bias]


def _run(inputs, trace=False, **kw):
    def f32(a):
        return np.ascontiguousarray(np.asarray(a), dtype=np.float32)

    x = f32(inputs["x"])
    W0, b0 = f32(inputs["W0"]), f32(inputs["b0"])
    W1, b1 = f32(inputs["W1"]), f32(inputs["b1"])
    W2, b2 = f32(inputs["W2"]), f32(inputs["b2"])
    Wc, bc = f32(inputs["Wc"]), f32(inputs["bc"])
    zero_bias = not (b0.any() or b1.any() or b2.any() or bc.any())
    nc = _get_nc(zero_bias)

    eye = np.eye(P, dtype=np.float32)
    in_maps = []
    for i in range(N_CORES):
        in_maps.append({
            "x": x[i * B:(i + 1) * B],
            "W0": W0, "b0": b0, "W1": W1, "b1": b1,
            "W2": W2, "b2": b2, "Wc": Wc, "bc": bc,
            "eye": eye,
        })
    res = run_bass_kernel_spmd(nc, in_maps, list(range(N_CORES)),
                               trace=trace, **kw)
    out = np.empty((B_FULL, 1), dtype=np.float32)
    for i in range(N_CORES):
        out[i * B:(i + 1) * B, 0] = res.results[i]["out"][0]
    return out, res


def kernel(**inputs) -> np.ndarray:
    out, _ = _run(inputs)
    return out

